# revision 1
# baseline (speedup 1.0000x reference)
"""AttentionGuidedDynamicRangeDWConv3D on 8 Trainium2 NeuronCores — v2.

Module: out = sum_i softmax(MLP(LN([mean_dhw(x), guidance])))[:, i]
                * dwconv3d(x, convw[i], convb[i], dil=i+1)
Shapes: x [4,96,16,56,56] f32, 3 branches of 3x3x3 depthwise conv with
dilations 1/2/3 ('same' zero padding).

Sharding: 8 cores = (batch b in 0..3) x (depth half h in 0..1); each core
owns 8 output planes and receives a host-padded 14-plane input slab.

v2 design:
- x is stored fp16 in a ROW-PADDED layout: each 56x56 plane row is padded
  to 64 columns and 3 zero rows are shared between consecutive planes
  (plane stride 59*64).  Every conv tap (od,oh,ow) then becomes an EXACT
  flat shift of this buffer: out-of-range reads land in zero padding, so
  no edge fix-ups are needed anywhere.
- The 81 taps (the 3 branch centers are merged into one effective center
  weight) are split across engines that run concurrently, with emission
  order matched to each engine's in-order instruction stream:
    PE   : 48 taps as diagonal fp16 matmuls, chunk-major over 7 compact
           448-col PSUM chunks per plane (each chunk's accumulation
           group closes early so the DVE merges overlap the PE).
    DVE  : 12 taps as self-paced 4x-mode tensor_scalar products plus
           2x-mode tensor_tensor adds into the accumulator; also adds
           the Act products, Pool group-sums and PSUM chunks.
    Act  : 18 taps as products via activation-copy with per-channel
           scale, consumed late in the plane so Act runs ahead.
    Pool : independent Act-product triple-sums, their adds interleaved
           among the singles so the group tiles recycle early.
  The gate MLP's cross-partition moves (feat transpose, row broadcasts,
  layer-2 column sums) are tiny PE matmuls against ones/identity tiles
  instead of DRAM round-trip DMAs.  Output is DMA'd out per plane as
  fp16 and cast to f32 on the host.

The gate MLP runs redundantly per core; the global pooled features need
one cross-core 384-float AllReduce.  The pooled sum itself is computed on
the PE (identity matmuls accumulating 512-col chunks into one PSUM bank)
so the startup critical path stays off the DVE.
"""

import copy
import sys

if "/opt/trn_rl_repo" not in sys.path:
    sys.path.insert(0, "/opt/trn_rl_repo")

import numpy as np

import concourse.bass as bass
import concourse.mybir as mybir
import concourse.tile as tile
from concourse.bass_utils import run_bass_kernel_spmd

F32 = mybir.dt.float32
F16 = mybir.dt.float16
ALU = mybir.AluOpType
ACTF = mybir.ActivationFunctionType

B, C, D, H, W = 4, 96, 16, 56, 56
G, HID, NB = 96, 24, 3
K = 3
DILS = (1, 2, 3)
LN_EPS = 1e-5
N_CORES = 8
DH = D // 2          # output planes per core
NPL = DH + 2 * 3     # local input planes incl. 3-deep halo
RS = 64              # padded row stride
PS = 59 * RS         # padded plane stride (56 rows + 3 shared pad rows)
XG = 224             # front guard (>= 3*RS + 3, zero-filled by host)
XGB = 224            # back guard
XTOT = XG + NPL * PS + XGB
PLANE = H * W        # compact output plane size
PPLANE = 56 * RS     # padded plane span covered by PE windows (3584)
NCH = 7              # 512-col psum chunks per padded plane
CH = 512

# tap split (sums to 78 = 81 - 3 merged centers)
N_PE = 48
N_DVE = 12
N_ACT = 18
N_POOL_PAIRS = 3     # independent Act-product group-sums computed on the Pool


def _tap_list():
    """All 81 (t, od, oh, ow); centers (od=oh=ow=0) listed separately."""
    taps, centers = [], []
    for i, dil in enumerate(DILS):
        for kd in range(K):
            for kh in range(K):
                for kw in range(K):
                    t = i * 27 + kd * 9 + kh * 3 + kw
                    e = (t, (kd - 1) * dil, (kh - 1) * dil, (kw - 1) * dil)
                    if e[1] == 0 and e[2] == 0 and e[3] == 0:
                        centers.append(e)
                    else:
                        taps.append(e)
    return taps, centers


def _build_program():
    nc = bass.Bass()
    xin = nc.dram_tensor("x", [C, XTOT], F16, kind="ExternalInput")
    gdin = nc.dram_tensor("gd", [G], F32, kind="ExternalInput")
    cwt_in = nc.dram_tensor("cwt", [C, NB * 27], F32, kind="ExternalInput")
    cbt_in = nc.dram_tensor("cbt", [C, NB], F32, kind="ExternalInput")
    w1t_in = nc.dram_tensor("w1t", [HID, C + G], F32, kind="ExternalInput")
    b1_in = nc.dram_tensor("b1", [HID], F32, kind="ExternalInput")
    w2_in = nc.dram_tensor("w2", [HID, NB], F32, kind="ExternalInput")
    b2_in = nc.dram_tensor("b2", [NB], F32, kind="ExternalInput")
    lng_in = nc.dram_tensor("lng", [C + G], F32, kind="ExternalInput")
    lnb_in = nc.dram_tensor("lnb", [C + G], F32, kind="ExternalInput")
    oh4_in = nc.dram_tensor("oh4", [C, B], F32, kind="ExternalInput")
    id_in = nc.dram_tensor("idp", [C, C], F16, kind="ExternalInput")
    yout = nc.dram_tensor("y", [C, DH * PLANE], F16, kind="ExternalOutput")

    last_writer = {}

    def dma_accum(out_ap, in_ap, key):
        """gpsimd accumulate-DMA with an explicit RMW ordering edge against
        the previous writer of the destination accumulator (the automatic
        tracker treats the dest as write-only)."""
        inst = nc.gpsimd.dma_start(out=out_ap, in_=in_ap, accum_op=ALU.add)
        prev = last_writer.get(key)
        if prev is not None:
            inst.ins.add_dependency(
                prev.ins.name if hasattr(prev, "ins") else prev.name,
                mybir.DependencyInfo(sync=True, no_sync=False),
            )
        last_writer[key] = inst
        return inst

    with tile.TileContext(nc) as tc:
        with (
            tc.tile_pool(name="sbuf", bufs=1) as pool,
            tc.tile_pool(name="diagp", bufs=1) as diagpool,
            tc.tile_pool(name="tmpp", bufs=6) as tmppool,
            tc.tile_pool(name="vtmp", bufs=2) as vtmppool,
            tc.tile_pool(name="pairp", bufs=3) as pairpool,
            tc.tile_pool(name="accp", bufs=3) as accpool,
            tc.tile_pool(name="dram", bufs=1, space="DRAM") as dpool,
            tc.tile_pool(name="psum", bufs=1, space="PSUM") as ppool,
        ):
            xbuf = pool.tile([C, XTOT], F16, tag="xbuf")
            w_eff = pool.tile([C, NB * 27], F32, tag="w_eff")
            w_ctr = pool.tile([C, 1], F32, tag="w_ctr")
            cwt = pool.tile([C, NB * 27], F32, tag="cwt")
            cbt = pool.tile([C, NB], F32, tag="cbt")
            b_eff = pool.tile([C, 1], F32, tag="b_eff")
            tmpb = pool.tile([C, NB], F32, tag="tmpb")
            onehot_bc = pool.tile([C, B], F32, tag="onehot_bc")
            featp = pool.tile([C, 1], F32, tag="featp")
            contrib = pool.tile([C, B], F32, tag="contrib")
            ar_s = pool.tile([C, B], F32, tag="ar_s")
            feat_full = pool.tile([C, 1], F32, tag="feat_full")
            g_row = pool.tile([1, C + G], F32, tag="g_row")
            gd_row = pool.tile([1, C + G], F32, tag="gd_row")
            lng = pool.tile([1, C + G], F32, tag="lng")
            lnb = pool.tile([1, C + G], F32, tag="lnb")
            gn_row = pool.tile([1, C + G], F32, tag="gn_row")
            w1t = pool.tile([HID, C + G], F32, tag="w1t")
            prod = pool.tile([HID, C + G], F32, tag="prod")
            hvec = pool.tile([HID, 1], F32, tag="hvec")
            b1c = pool.tile([HID, 1], F32, tag="b1c")
            w2t = pool.tile([HID, NB], F32, tag="w2t")
            l2tmp = pool.tile([HID, NB], F32, tag="l2tmp")
            zrow = pool.tile([1, NB], F32, tag="zrow")
            b2r = pool.tile([1, NB], F32, tag="b2r")
            wts = pool.tile([1, NB], F32, tag="wts")
            wts_bc = pool.tile([C, NB], F32, tag="wts_bc")
            idp = pool.tile([C, C], F16, tag="idp")
            idf32 = pool.tile([C, C], F32, tag="idf32")
            ones1c = pool.tile([1, C], F32, tag="ones1c")
            ones1h = pool.tile([1, HID], F32, tag="ones1h")
            ones_h1 = pool.tile([HID, 1], F32, tag="ones_h1")
            s1 = pool.tile([1, 1], F32, tag="s1")
            s2 = pool.tile([1, 1], F32, tag="s2")
            s3 = pool.tile([1, 1], F32, tag="s3")
            s4 = pool.tile([1, 1], F32, tag="s4")

            cin = dpool.tile([C, B], F32, tag="cin")
            cout = dpool.tile([C, B], F32, tag="cout")

            v = nc.vector
            sc = nc.scalar
            v.memset(ones1c[:, :], 1.0)
            v.memset(ones1h[:, :], 1.0)
            v.memset(ones_h1[:, :], 1.0)

            # ---- loads (owned slab first so pooling can start early) ----
            o0 = XG + 3 * PS
            o1 = XG + (3 + DH) * PS
            mid = XG + 7 * PS
            # owned slab + idp first: the startup-critical pooling matmuls
            # need exactly these (DMA transfers serialize, so queue order
            # controls when pooling can start); halo planes are not needed
            # until the first conv tap and go last
            nc.sync.dma_start(out=xbuf[:, o0:mid], in_=xin[:, o0:mid])
            nc.sync.dma_start(out=xbuf[:, mid:o1], in_=xin[:, mid:o1])
            nc.sync.dma_start(out=idp[:, :], in_=id_in[:, :])
            v.tensor_scalar_mul(idf32[:, :], idp[:, :], 1.0)
            nc.sync.dma_start(out=cwt[:, :], in_=cwt_in[:, :])
            nc.sync.dma_start(out=cbt[:, :], in_=cbt_in[:, :])
            nc.sync.dma_start(out=w1t[:, :], in_=w1t_in[:, :])
            nc.sync.dma_start(out=b1c[:, :], in_=b1_in[:, None])
            nc.sync.dma_start(out=w2t[:, :], in_=w2_in[:, :])
            nc.sync.dma_start(out=b2r[:, :], in_=b2_in[None, :])
            nc.sync.dma_start(out=lng[:, :], in_=lng_in[None, :])
            nc.sync.dma_start(out=lnb[:, :], in_=lnb_in[None, :])
            nc.sync.dma_start(out=onehot_bc[:, :], in_=oh4_in[:, :])
            nc.sync.dma_start(out=g_row[:, C:], in_=gdin[None, :])
            nc.sync.dma_start(out=xbuf[:, :o0], in_=xin[:, :o0])
            nc.sync.dma_start(out=xbuf[:, o1:], in_=xin[:, o1:])

            # ---- global-pool partial over owned planes, on the PE ----
            # sum of 59 x 512-col chunks (pads are zero) into one PSUM bank
            pps = ppool.tile([C, CH], F32, tag="ps0")
            nchunk = DH * PS // CH  # 59
            for k2 in range(nchunk):
                nc.tensor.matmul(
                    pps[:, :],
                    idp[:, :],
                    xbuf[:, o0 + k2 * CH : o0 + (k2 + 1) * CH],
                    start=(k2 == 0),
                    stop=False,
                    skip_group_check=True,
                )
            v.reduce_sum(featp[:, :], pps[:, :], axis=mybir.AxisListType.X)
            v.tensor_scalar_mul(featp[:, :], featp[:, :], 1.0 / (D * PLANE))
            v.tensor_scalar(
                out=contrib[:, :], in0=onehot_bc[:, :], scalar1=featp[:, :],
                scalar2=None, op0=ALU.mult,
            )

            # ---- cross-core AllReduce of [C, B] partials ----
            nc.sync.dma_start(out=cin[:, :], in_=contrib[:, :])
            nc.gpsimd.collective_compute(
                "AllReduce",
                ALU.add,
                replica_groups=[list(range(N_CORES))],
                ins=[cin.opt()],
                outs=[cout.opt()],
            )
            nc.sync.dma_start(out=ar_s[:, :], in_=cout[:, :])
            v.tensor_tensor(out=ar_s[:, :], in0=ar_s[:, :], in1=onehot_bc[:, :], op=ALU.mult)
            v.reduce_sum(feat_full[:, :], ar_s[:, :], axis=mybir.AxisListType.X)

            # ---- transpose feat to a single-partition row on the PE
            ps_t = ppool.tile([1, C], F32, tag="ps1")
            nc.tensor.transpose(ps_t[:, :], feat_full[:, :], idf32[:, :])
            v.tensor_scalar_mul(g_row[:, :C], ps_t[:, :], 1.0)

            # ---- LayerNorm over 192 on one partition ----
            v.reduce_sum(s1[:, :], g_row[:, :], axis=mybir.AxisListType.X)
            v.tensor_scalar_mul(s1[:, :], s1[:, :], 1.0 / (C + G))  # mu
            v.tensor_scalar(
                out=gd_row[:, :], in0=g_row[:, :], scalar1=s1[:, :], scalar2=None,
                op0=ALU.subtract,
            )
            v.tensor_tensor(out=gn_row[:, :], in0=gd_row[:, :], in1=gd_row[:, :], op=ALU.mult)
            v.reduce_sum(s2[:, :], gn_row[:, :], axis=mybir.AxisListType.X)
            v.tensor_scalar(
                out=s2[:, :], in0=s2[:, :], scalar1=1.0 / (C + G), scalar2=LN_EPS,
                op0=ALU.mult, op1=ALU.add,
            )  # var + eps
            sc.activation(s3[:, :], s2[:, :], ACTF.Sqrt)
            # one Newton step for a clean sqrt
            v.reciprocal(s4[:, :], s3[:, :])
            v.tensor_tensor(out=s4[:, :], in0=s4[:, :], in1=s2[:, :], op=ALU.mult)
            v.tensor_tensor(out=s4[:, :], in0=s4[:, :], in1=s3[:, :], op=ALU.add)
            v.tensor_scalar_mul(s4[:, :], s4[:, :], 0.5)
            v.reciprocal(s3[:, :], s4[:, :])  # rstd
            v.tensor_scalar(
                out=gn_row[:, :], in0=gd_row[:, :], scalar1=s3[:, :], scalar2=None,
                op0=ALU.mult,
            )
            v.tensor_tensor(out=gn_row[:, :], in0=gn_row[:, :], in1=lng[:, :], op=ALU.mult)
            v.tensor_tensor(out=gn_row[:, :], in0=gn_row[:, :], in1=lnb[:, :], op=ALU.add)

            # ---- MLP layer 1: h = gelu(gn @ w1 + b1) via row-products;
            # gn broadcast across partitions via a tiny PE ones-matmul
            ps_g = ppool.tile([HID, C + G], F32, tag="ps2")
            nc.tensor.matmul(ps_g[:, :], ones1h[:, :], gn_row[:, :],
                             start=True, stop=True, skip_group_check=True)
            v.tensor_tensor(out=prod[:, :], in0=w1t[:, :], in1=ps_g[:, :], op=ALU.mult)
            v.reduce_sum(hvec[:, :], prod[:, :], axis=mybir.AxisListType.X)
            v.tensor_tensor(out=hvec[:, :], in0=hvec[:, :], in1=b1c[:, :], op=ALU.add)
            sc.activation(hvec[:, :], hvec[:, :], ACTF.Gelu)

            # ---- MLP layer 2 via DRAM transpose bounce ----
            v.tensor_scalar(
                out=l2tmp[:, :], in0=w2t[:, :], scalar1=hvec[:, :], scalar2=None,
                op0=ALU.mult,
            )
            ps_z = ppool.tile([1, NB], F32, tag="ps3")
            nc.tensor.matmul(ps_z[:, :], ones_h1[:, :], l2tmp[:, :],
                             start=True, stop=True, skip_group_check=True)
            v.tensor_tensor(out=zrow[:, :], in0=ps_z[:, :], in1=b2r[:, :], op=ALU.add)

            # ---- softmax over 3 ----
            v.reduce_max(s1[:, :], zrow[:, :], axis=mybir.AxisListType.X)
            v.tensor_scalar(
                out=zrow[:, :], in0=zrow[:, :], scalar1=s1[:, :], scalar2=None,
                op0=ALU.subtract,
            )
            sc.activation(zrow[:, :], zrow[:, :], ACTF.Exp)
            v.reduce_sum(s2[:, :], zrow[:, :], axis=mybir.AxisListType.X)
            v.reciprocal(s2[:, :], s2[:, :])
            v.tensor_scalar(
                out=wts[:, :], in0=zrow[:, :], scalar1=s2[:, :], scalar2=None,
                op0=ALU.mult,
            )

            # ---- fold gate weights into per-tap channel weights ----
            ps_wb = ppool.tile([C, NB], F32, tag="ps4")
            nc.tensor.matmul(ps_wb[:, :], ones1c[:, :], wts[:, :],
                             start=True, stop=True, skip_group_check=True)
            v.tensor_scalar_mul(wts_bc[:, :], ps_wb[:, :], 1.0)
            for i in range(NB):
                v.tensor_scalar(
                    out=w_eff[:, i * 27 : (i + 1) * 27],
                    in0=cwt[:, i * 27 : (i + 1) * 27],
                    scalar1=wts_bc[:, i : i + 1],
                    scalar2=None,
                    op0=ALU.mult,
                )
            v.tensor_tensor(out=tmpb[:, :], in0=cbt[:, :], in1=wts_bc[:, :], op=ALU.mult)
            v.reduce_sum(b_eff[:, :], tmpb[:, :], axis=mybir.AxisListType.X)
            # merged center weight = sum of the 3 branch centers
            ctrs = [i * 27 + 13 for i in range(NB)]
            v.tensor_tensor(
                out=w_ctr[:, :], in0=w_eff[:, ctrs[0] : ctrs[0] + 1],
                in1=w_eff[:, ctrs[1] : ctrs[1] + 1], op=ALU.add,
            )
            v.tensor_tensor(
                out=w_ctr[:, :], in0=w_ctr[:, :],
                in1=w_eff[:, ctrs[2] : ctrs[2] + 1], op=ALU.add,
            )

            # ---- the conv ----
            taps, _ = _tap_list()
            pe_taps = taps[:N_PE]
            dve_taps = taps[N_PE : N_PE + N_DVE]
            act_taps = taps[N_PE + N_DVE :]
            assert len(act_taps) == N_ACT

            # all PE diagonals built once up front (DVE 4x tensor_scalar,
            # ~85ns each) so the PE never waits on the Act engine
            diags = {}
            for t, od, oh, ow in pe_taps:
                dg = diagpool.tile([C, C], F16, tag=f"dg{t}")
                v.tensor_scalar(
                    out=dg[:, :], in0=idp[:, :], scalar1=w_eff[:, t : t + 1],
                    scalar2=None, op0=ALU.mult,
                )
                diags[t] = dg

            def win(p, od, oh, ow):
                """flat-shifted padded window of the input for one tap"""
                base = XG + (p + od) * PS + oh * RS + ow
                return base

            def win3d(p, od, oh, ow):
                base = win(p, od, oh, ow)
                return xbuf[:, base : base + PPLANE].rearrange(
                    "c (h w) -> c h w", h=H, w=RS
                )[:, :, 0:W]

            CCH = 448  # compact psum chunk (8 output rows)
            for p in range(3, 3 + DH):
                pli = p - 3
                a0 = accpool.tile([C, PLANE], F16, tag="a0")
                a0v = a0[:, :].rearrange("c (h w) -> c h w", h=H, w=W)

                # --- PE: diagonal matmuls; strided rhs windows skip the pad
                # columns so psum chunks stay compact (448 = 8 output rows)
                pss = []
                for ci in range(NCH):
                    ps = ppool.tile([C, CCH], F32, tag=f"ps{(pli * NCH + ci) % 8}")
                    pss.append(ps)
                # chunk-major: each PSUM chunk's accumulation group closes
                # after its 50 matmuls (~9us), so the DVE merge of chunk ci
                # overlaps the PE working on chunk ci+1
                for ci in range(NCH):
                    for tn, (t, od, oh, ow) in enumerate(pe_taps):
                        base = win(p, od, oh, ow)
                        rhs = xbuf[:, base + ci * 8 * RS : base + (ci + 1) * 8 * RS]
                        rhs = rhs.rearrange("c (h w) -> c h w", h=8, w=RS)[:, :, 0:W]
                        nc.tensor.matmul(
                            pss[ci][:, :],
                            diags[t][:, :],
                            rhs,
                            start=(tn == 0),
                            stop=False,
                            skip_group_check=True,
                        )

                # --- A0 seed: merged center tap + bias
                v.tensor_scalar(
                    out=a0v, in0=win3d(p, 0, 0, 0), scalar1=w_ctr[:, :],
                    scalar2=b_eff[:, :], op0=ALU.mult, op1=ALU.add,
                )

                def merge_chunk(ci):
                    v.tensor_tensor(
                        out=a0[:, ci * CCH : (ci + 1) * CCH],
                        in0=a0[:, ci * CCH : (ci + 1) * CCH],
                        in1=pss[ci][:, :],
                        op=ALU.add,
                    )

                # --- DVE taps: 4x-mode product + 2x-mode add, both on the
                # DVE itself (self-paced, no cross-engine waits).  PSUM chunk
                # merges are interleaved right after each chunk's matmul
                # group closes so the banks recycle early and the PE never
                # waits on them.
                next_merge = [0]
                merge_after_tap = {3: 0, 7: 1, 11: 2}
                for tn2, (t, od, oh, ow) in enumerate(dve_taps):
                    vt = vtmppool.tile([C, PLANE], F16, tag="vtmp")
                    v.tensor_scalar(
                        out=vt[:, :].rearrange("c (h w) -> c h w", h=H, w=W),
                        in0=win3d(p, od, oh, ow),
                        scalar1=w_eff[:, t : t + 1], scalar2=None, op0=ALU.mult,
                    )
                    v.tensor_tensor(out=a0[:, :], in0=a0[:, :], in1=vt[:, :], op=ALU.add)
                    if tn2 in merge_after_tap:
                        merge_chunk(merge_after_tap[tn2])
                        next_merge[0] = merge_after_tap[tn2] + 1

                def emit_act_prod(tap):
                    t, od, oh, ow = tap
                    tmp = tmppool.tile([C, PLANE], F16, tag="tmp")
                    tv = tmp[:, :].rearrange("c (h w) -> c h w", h=H, w=W)
                    sc.activation(tv, win3d(p, od, oh, ow), ACTF.Copy,
                                  scale=w_eff[:, t : t + 1])
                    return tmp

                # Act products: pair-batch the first 2*N_POOL_PAIRS (Pool sums
                # each pair, freeing their tmp slots quickly), DVE adds the
                # rest late in the plane when Act has run ahead.
                groups = [3, 3, 3, 2][:N_POOL_PAIRS] if N_POOL_PAIRS >= 3 else [2] * N_POOL_PAIRS
                n_grouped = sum(groups)
                pairsums = []
                gi = 0
                for gsz in groups:
                    t1 = emit_act_prod(act_taps[gi])
                    t2 = emit_act_prod(act_taps[gi + 1])
                    s = pairpool.tile([C, PLANE], F16, tag="pairsum")
                    nc.gpsimd.tensor_tensor(
                        out=s[:, :], in0=t1[:, :], in1=t2[:, :], op=ALU.add
                    )
                    if gsz == 3:
                        t3 = emit_act_prod(act_taps[gi + 2])
                        nc.gpsimd.tensor_tensor(
                            out=s[:, :], in0=s[:, :], in1=t3[:, :], op=ALU.add
                        )
                    pairsums.append(s)
                    gi += gsz
                # interleave the group-sum adds among the singles so each
                # pairsum slot frees early (Act needs it for the next plane)
                singles = act_taps[n_grouped:]
                gslots = {}
                if pairsums:
                    step = max(1, len(singles) // len(pairsums))
                    for gi2, s in enumerate(pairsums):
                        gslots[min((gi2 + 1) * step, len(singles))] = s
                merge_after_single = {2: 3, 6: 4, 10: 5}
                nadds = [0]
                for k2, tap in enumerate(singles):
                    tmp = emit_act_prod(tap)
                    v.tensor_tensor(out=a0[:, :], in0=a0[:, :], in1=tmp[:, :], op=ALU.add)
                    nadds[0] += 1
                    if nadds[0] in merge_after_single:
                        merge_chunk(merge_after_single[nadds[0]])
                        next_merge[0] = merge_after_single[nadds[0]] + 1
                    if (k2 + 1) in gslots:
                        v.tensor_tensor(out=a0[:, :], in0=a0[:, :],
                                        in1=gslots[k2 + 1][:, :], op=ALU.add)
                        nadds[0] += 1
                        if nadds[0] in merge_after_single:
                            merge_chunk(merge_after_single[nadds[0]])
                            next_merge[0] = merge_after_single[nadds[0]] + 1

                # --- any remaining psum merges (at least the last chunk)
                for ci in range(next_merge[0], NCH):
                    merge_chunk(ci)
                nc.sync.dma_start(
                    out=yout[:, pli * PLANE : (pli + 1) * PLANE], in_=a0[:, :]
                )

    _split_sem_waits(nc)
    return nc


_WAITSPLIT = [0]


def _split_sem_waits(nc, max_waits=1):
    """This walrus build rejects >1 SyncWait per instruction (and any wait on
    a Drain). Move excess waits onto same-engine NOPs inserted just before."""
    for bb in nc.main_func.blocks:
        insns = bb.instructions
        i = 0
        while i < len(insns):
            ins = insns[i]
            si = ins.sync_info
            limit = 0 if ins.opcode == "Drain" else max_waits
            if si is not None and si.on_wait is not None and len(si.on_wait) > limit:
                waits = list(si.on_wait)
                keep = waits[-limit:] if limit else []
                extra = waits[: len(waits) - limit]
                pos = i
                for j in range(0, len(extra), max_waits):
                    nop = mybir.InstNoOp(
                        name=f"I-waitsplit-{_WAITSPLIT[0]}", ins=[], outs=[]
                    )
                    _WAITSPLIT[0] += 1
                    nop.engine = ins.engine
                    nop.sync_info = mybir.SyncInfo(
                        on_wait=extra[j : j + max_waits], on_update=[]
                    )
                    insns.insert(pos, nop)
                    pos += 1
                    i += 1
                si.on_wait = keep
            i += 1


def _prep_inputs(x, guidance, convw, convb, ln_g, ln_b, w1, b1, w2, b2):
    f = np.float32
    cwt = np.ascontiguousarray(
        convw.reshape(NB, C, 27).transpose(1, 0, 2).reshape(C, NB * 27), dtype=f
    )
    cbt = np.ascontiguousarray(convb.T, dtype=f)
    w1t = np.ascontiguousarray(w1.T, dtype=f)
    idp = np.eye(C, dtype=np.float16)
    common = dict(
        cwt=cwt, cbt=cbt, w1t=w1t,
        b1=np.ascontiguousarray(b1, dtype=f),
        w2=np.ascontiguousarray(w2, dtype=f),
        b2=np.ascontiguousarray(b2, dtype=f),
        lng=np.ascontiguousarray(ln_g, dtype=f),
        lnb=np.ascontiguousarray(ln_b, dtype=f),
        idp=idp,
    )
    x16 = x.astype(np.float16)
    in_maps = []
    for core in range(N_CORES):
        b, hh = core // 2, core % 2
        lo = 8 * hh - 3
        buf = np.zeros((C, XTOT), dtype=np.float16)
        view = buf[:, XG : XG + NPL * PS].reshape(C, NPL, 59, RS)
        g0, g1 = max(0, lo), min(D, lo + NPL)
        view[:, g0 - lo : g1 - lo, :H, :W] = x16[b, :, g0:g1]
        onehot = np.zeros((C, B), dtype=f)
        onehot[:, b] = 1.0
        in_maps.append(
            dict(
                x=buf,
                gd=np.ascontiguousarray(guidance[b], dtype=f),
                oh4=onehot,
                **common,
            )
        )
    return in_maps


_CACHED_NC = None


def kernel(x, guidance, convw, convb, ln_g, ln_b, w1, b1, w2, b2):
    global _CACHED_NC
    if _CACHED_NC is None:
        _CACHED_NC = _build_program()
    in_maps = _prep_inputs(
        x, guidance, convw, convb, ln_g, ln_b, w1, b1, w2, b2
    )
    res = run_bass_kernel_spmd(_CACHED_NC, in_maps, list(range(N_CORES)))
    out = np.empty((B, C, D, H, W), dtype=np.float32)
    for core in range(N_CORES):
        b, hh = core // 2, core % 2
        out[b, :, 8 * hh : 8 * hh + 8] = (
            res.results[core]["y"].astype(np.float32).reshape(C, DH, H, W)
        )
    return out


if __name__ == "__main__":
    rng = np.random.default_rng(0)
    ins = dict(
        x=rng.standard_normal((B, C, D, H, W), dtype=np.float32),
        guidance=rng.standard_normal((B, G), dtype=np.float32),
        convw=(rng.standard_normal((NB, C, 1, K, K, K)) * 0.1).astype(np.float32),
        convb=np.zeros((NB, C), np.float32),
        ln_g=np.ones((C + G,), np.float32),
        ln_b=np.zeros((C + G,), np.float32),
        w1=(rng.standard_normal((C + G, HID)) * 0.05).astype(np.float32),
        b1=np.zeros((HID,), np.float32),
        w2=(rng.standard_normal((NB,)) * 0.05).astype(np.float32),
        b2=np.zeros((NB,), np.float32),
    )
    out = kernel(**ins)
    print("kernel ran, out shape", out.shape)



# revision 12
# speedup vs baseline: 1.0087x; 1.0087x over previous
"""AttentionGuidedDynamicRangeDWConv3D on 8 Trainium2 NeuronCores — v3.

Module: out = sum_i softmax(MLP(LN([mean_dhw(x), guidance])))[:, i]
                * dwconv3d(x, convw[i], convb[i], dil=i+1)
Shapes: x [4,96,16,56,56] f32, 3 branches of 3x3x3 depthwise conv with
dilations 1/2/3 ('same' zero padding).

Sharding: 8 cores = (batch b in 0..3) x (depth half h in 0..1); each core
owns 8 output planes.

v3 design (vs v2):
- No collective: each core loads a 14-plane padded fp16 slab (owned 8 +
  3+3 halo planes, host-zero-filled where out of range) plus the other 5
  planes of its batch compactly, so the gate's global pool is computed
  locally (PE identity-matmul pooling over the slab + Act accum_out
  pooling over the compact planes). The slab layout is identical for
  both depth-halves, so one program serves all 8 SPMD cores.
- Row-padded layout with RS=60 (56 data + 4 shared pad cols), 3 shared
  pad rows between planes; every in-range tap is an exact flat shift.
- Partition 96 of the slab is all-ones: the first PE matmul of every
  PSUM chunk uses a 97-row lhsT whose row 96 is b_eff, folding the conv
  bias into the PE pass for free.
- Tap work is split across all five devices at the cost-model balance
  point (per full plane): 38 taps on PE (diagonal fp16 matmuls into 7
  448-col PSUM chunks), 23 DVE tensor_scalar products (3 seed chain
  accumulators c1..c3 directly, 20 go to tmp tiles that gpsimd
  accumulate-DMAs add into the chains), 14 Act products added into a c0
  accumulator by DVE tensor_tensor (first seeds c0), and 3 Act products
  routed into the DMA chains. Accum-DMAs into one destination must be
  serialized (HW RMW races otherwise), so three independent chains keep
  the DMA engines saturated while each chain's links wait on sems.
- PSUM chunks merge into the fp16 accumulator a0 via DVE
  scalar_tensor_tensor (center-tap product fused with the PSUM add);
  c0/c3 merge into a0 on the Pool engine; a0 is DMA'd to DRAM y as the
  seed and chains c1/c2 are accumulate-DMA'd on top.
"""

import sys

if "/opt/trn_rl_repo" not in sys.path:
    sys.path.insert(0, "/opt/trn_rl_repo")

import numpy as np

import concourse.bass as bass
import concourse.mybir as mybir
import concourse.tile as tile
from concourse.bass_utils import run_bass_kernel_spmd

F32 = mybir.dt.float32
F16 = mybir.dt.float16
ALU = mybir.AluOpType
ACTF = mybir.ActivationFunctionType

B, C, D, H, W = 4, 96, 16, 56, 56
G, HID, NB = 96, 24, 3
K = 3
DILS = (1, 2, 3)
LN_EPS = 1e-5
N_CORES = 8
DH = D // 2          # output planes per core
NSLAB = 14           # padded slab planes (8 owned + 3+3 halo)
S_OF = 3             # slab index of first owned plane
NEXTRA = 5           # compact planes for pooling only
RS = 60              # padded row stride (56 data + 4 pad)
PR = 59              # rows per plane incl 3 shared pad rows
PS = PR * RS         # padded plane stride
XG = 192             # front guard (>= 3*RS + 3, zero-filled by host)
XGB = 64             # back guard (window row-slices overrun by <= 3 cols)
XTOT = XG + NSLAB * PS + XGB
PLANE = H * W
CCH = 448            # compact psum chunk (8 output rows)
NCH = 7
INV_CNT = 1.0 / (D * H * W)

QUOTAS = {"pe": 38, "da": 23, "ad": 14, "aa": 3}


def _tap_list():
    """All 81 (t, od, oh, ow); centers (0,0,0) listed separately."""
    taps, centers = [], []
    for i, dil in enumerate(DILS):
        for kd in range(K):
            for kh in range(K):
                for kw in range(K):
                    t = i * 27 + kd * 9 + kh * 3 + kw
                    e = (t, (kd - 1) * dil, (kh - 1) * dil, (kw - 1) * dil)
                    if e[1] == 0 and e[2] == 0 and e[3] == 0:
                        centers.append(e)
                    else:
                        taps.append(e)
    return taps, centers


def _assign_taps():
    """Split the 78 non-center taps into pe/da/ad/aa lists, keeping each
    od-class proportionally represented (D-edge clipping then thins every
    engine evenly) and putting od=0 taps first in each list (chain seeds
    and the PE bias carrier must never be clipped)."""
    taps, _ = _tap_list()
    byod = {}
    for e in taps:
        byod.setdefault(e[1], []).append(e)
    classes = sorted(byod, key=lambda od: (od != 0, abs(od), od))
    out = {k: [] for k in QUOTAS}
    remaining = dict(QUOTAS)
    nleft = len(taps)
    for od in classes:
        grp = byod[od]
        share = {k: remaining[k] * len(grp) / nleft for k in QUOTAS}
        base = {k: int(share[k]) for k in QUOTAS}
        extra = len(grp) - sum(base.values())
        for k in sorted(QUOTAS, key=lambda k: share[k] - base[k], reverse=True)[:extra]:
            base[k] += 1
        i = 0
        for k in ("pe", "da", "ad", "aa"):
            out[k].extend(grp[i : i + base[k]])
            i += base[k]
            remaining[k] -= base[k]
        nleft -= len(grp)
    assert all(len(out[k]) == QUOTAS[k] for k in QUOTAS)
    assert out["pe"][0][1] == 0 and out["ad"][0][1] == 0
    assert all(e[1] == 0 for e in out["da"][:3])
    return out


def _build_program():
    """One program for all 8 SPMD cores; per-core variation is entirely in
    the host-prepared input tensors (slab content + guidance)."""
    nc = bass.Bass()
    xin = nc.dram_tensor("x", [C + 1, XTOT], F16, kind="ExternalInput")
    xein = nc.dram_tensor("xe", [C, NEXTRA * PLANE], F16, kind="ExternalInput")
    gdin = nc.dram_tensor("gd", [G], F32, kind="ExternalInput")
    cwt_in = nc.dram_tensor("cwt", [C, NB * 27], F32, kind="ExternalInput")
    cbt_in = nc.dram_tensor("cbt", [C, NB], F32, kind="ExternalInput")
    w1t_in = nc.dram_tensor("w1t", [HID, C + G], F32, kind="ExternalInput")
    b1_in = nc.dram_tensor("b1", [HID], F32, kind="ExternalInput")
    w2_in = nc.dram_tensor("w2", [HID, NB], F32, kind="ExternalInput")
    b2_in = nc.dram_tensor("b2", [NB], F32, kind="ExternalInput")
    lng_in = nc.dram_tensor("lng", [C + G], F32, kind="ExternalInput")
    lnb_in = nc.dram_tensor("lnb", [C + G], F32, kind="ExternalInput")
    id_in = nc.dram_tensor("idp", [C + 1, C], F16, kind="ExternalInput")
    yout = nc.dram_tensor("y", [C, DH * PLANE], F16, kind="ExternalOutput")

    asn = _assign_taps()

    def dep(inst, prev):
        inst.ins.add_dependency(
            prev.ins.name if hasattr(prev, "ins") else prev.name,
            mybir.DependencyInfo(sync=True, no_sync=False),
        )

    with tile.TileContext(nc) as tc:
        with (
            tc.tile_pool(name="sbuf", bufs=1) as pool,
            tc.tile_pool(name="diagp", bufs=1) as diagpool,
            tc.tile_pool(name="tmpdp", bufs=3) as tmpdpool,
            tc.tile_pool(name="tmpap", bufs=3) as tmpapool,
            tc.tile_pool(name="chp", bufs=2) as chpool,
            tc.tile_pool(name="psum", bufs=1, space="PSUM") as ppool,
        ):
            xbuf = pool.tile([C + 1, XTOT], F16, tag="xbuf")
            w_eff = pool.tile([C + 1, NB * 27], F32, tag="w_eff")
            w_ctr = pool.tile([C, 1], F32, tag="w_ctr")
            cwt = pool.tile([C, NB * 27], F32, tag="cwt")
            cbt = pool.tile([C, NB], F32, tag="cbt")
            b_eff = pool.tile([C, 1], F32, tag="b_eff")
            tmpb = pool.tile([C, NB], F32, tag="tmpb")
            featp = pool.tile([C, 1], F32, tag="featp")
            pacc = pool.tile([C, NEXTRA], F32, tag="pacc")
            pacs = pool.tile([C, 1], F32, tag="pacs")
            g_row = pool.tile([1, C + G], F32, tag="g_row")
            gd_row = pool.tile([1, C + G], F32, tag="gd_row")
            lng = pool.tile([1, C + G], F32, tag="lng")
            lnb = pool.tile([1, C + G], F32, tag="lnb")
            gn_row = pool.tile([1, C + G], F32, tag="gn_row")
            w1t = pool.tile([HID, C + G], F32, tag="w1t")
            prod = pool.tile([HID, C + G], F32, tag="prod")
            hvec = pool.tile([HID, 1], F32, tag="hvec")
            b1c = pool.tile([HID, 1], F32, tag="b1c")
            w2t = pool.tile([HID, NB], F32, tag="w2t")
            l2tmp = pool.tile([HID, NB], F32, tag="l2tmp")
            zrow = pool.tile([1, NB], F32, tag="zrow")
            b2r = pool.tile([1, NB], F32, tag="b2r")
            wts = pool.tile([1, NB], F32, tag="wts")
            wts_bc = pool.tile([C, NB], F32, tag="wts_bc")
            idp = pool.tile([C + 1, C], F16, tag="idp")
            idf32 = pool.tile([C, C], F32, tag="idf32")
            ones1c = pool.tile([1, C], F32, tag="ones1c")
            ones1h = pool.tile([1, HID], F32, tag="ones1h")
            ones_h1 = pool.tile([HID, 1], F32, tag="ones_h1")
            s1 = pool.tile([1, 1], F32, tag="s1")
            s2 = pool.tile([1, 1], F32, tag="s2")
            s3 = pool.tile([1, 1], F32, tag="s3")
            s4 = pool.tile([1, 1], F32, tag="s4")
            a0 = pool.tile([C, PLANE], F16, tag="a0")

            v = nc.vector
            sc = nc.scalar
            v.memset(ones1c[:, :], 1.0)
            v.memset(ones1h[:, :], 1.0)
            v.memset(ones_h1[:, :], 1.0)

            # ---- loads: idp + small weights first (tiny), then the 5
            # compact pool planes (Act pooling starts early), then the slab
            # in pieces (PE pooling chunks chase the pieces) ----
            nc.sync.dma_start(out=idp[:, :], in_=id_in[:, :])
            nc.sync.dma_start(out=cwt[:, :], in_=cwt_in[:, :])
            nc.sync.dma_start(out=cbt[:, :], in_=cbt_in[:, :])
            nc.sync.dma_start(out=w1t[:, :], in_=w1t_in[:, :])
            nc.sync.dma_start(out=b1c[:, :], in_=b1_in[:, None])
            nc.sync.dma_start(out=w2t[:, :], in_=w2_in[:, :])
            nc.sync.dma_start(out=b2r[:, :], in_=b2_in[None, :])
            nc.sync.dma_start(out=lng[:, :], in_=lng_in[None, :])
            nc.sync.dma_start(out=lnb[:, :], in_=lnb_in[None, :])
            nc.sync.dma_start(out=g_row[:, C:], in_=gdin[None, :])
            v.tensor_scalar_mul(idf32[:, :], idp[0:C, :], 1.0)

            xe_tiles = []
            for i in range(NEXTRA):
                xe = tmpapool.tile([C, PLANE], F16, tag="tmpA")
                nc.sync.dma_start(
                    out=xe[:, :], in_=xein[:, i * PLANE : (i + 1) * PLANE]
                )
                xe_tiles.append(xe)
            npiece = 4
            bounds = [XG + (NSLAB * k // npiece) * PS for k in range(npiece + 1)]
            bounds[0] = 0
            for k in range(npiece):
                nc.sync.dma_start(
                    out=xbuf[:, bounds[k] : bounds[k + 1]],
                    in_=xin[:, bounds[k] : bounds[k + 1]],
                )

            # ---- global pool: PE identity matmuls over the slab ----
            pps = ppool.tile([C, 512], F32, tag="pool")
            slab_cols = NSLAB * PS
            nchunk = (slab_cols + 511) // 512
            for k in range(nchunk):
                c0_ = XG + k * 512
                wdt = min(512, XG + slab_cols - c0_)
                nc.tensor.matmul(
                    pps[:, 0:wdt], idp[0:C, :], xbuf[0:C, c0_ : c0_ + wdt],
                    start=(k == 0), stop=False, skip_group_check=True,
                )
            # ---- Act accum_out pooling over the 5 compact planes ----
            for i, xe in enumerate(xe_tiles):
                junk = tmpdpool.tile([C, PLANE], F16, tag="tmpD")
                sc.activation(junk[:, :], xe[:, :], ACTF.Copy,
                              accum_out=pacc[:, i : i + 1])
            v.reduce_sum(featp[:, :], pps[:, :], axis=mybir.AxisListType.X)
            v.reduce_sum(pacs[:, :], pacc[:, :], axis=mybir.AxisListType.X)
            v.tensor_tensor(out=featp[:, :], in0=featp[:, :], in1=pacs[:, :], op=ALU.add)
            v.tensor_scalar_mul(featp[:, :], featp[:, :], INV_CNT)

            # ---- transpose feat to one row (PE), reusing the pool bank ----
            ps_t = pps[0:1, 0:96]
            nc.tensor.transpose(ps_t, featp[:, :], idf32[:, :])
            v.tensor_scalar_mul(g_row[:, :C], ps_t, 1.0)

            # ---- LayerNorm over 192 on one partition ----
            v.reduce_sum(s1[:, :], g_row[:, :], axis=mybir.AxisListType.X)
            v.tensor_scalar_mul(s1[:, :], s1[:, :], 1.0 / (C + G))  # mu
            v.tensor_scalar(
                out=gd_row[:, :], in0=g_row[:, :], scalar1=s1[:, :], scalar2=None,
                op0=ALU.subtract,
            )
            v.tensor_tensor(out=gn_row[:, :], in0=gd_row[:, :], in1=gd_row[:, :], op=ALU.mult)
            v.reduce_sum(s2[:, :], gn_row[:, :], axis=mybir.AxisListType.X)
            v.tensor_scalar(
                out=s2[:, :], in0=s2[:, :], scalar1=1.0 / (C + G), scalar2=LN_EPS,
                op0=ALU.mult, op1=ALU.add,
            )  # var + eps
            sc.activation(s3[:, :], s2[:, :], ACTF.Sqrt)
            v.reciprocal(s4[:, :], s3[:, :])
            v.tensor_tensor(out=s4[:, :], in0=s4[:, :], in1=s2[:, :], op=ALU.mult)
            v.tensor_tensor(out=s4[:, :], in0=s4[:, :], in1=s3[:, :], op=ALU.add)
            v.tensor_scalar_mul(s4[:, :], s4[:, :], 0.5)
            v.reciprocal(s3[:, :], s4[:, :])  # rstd
            v.tensor_scalar(
                out=gn_row[:, :], in0=gd_row[:, :], scalar1=s3[:, :], scalar2=None,
                op0=ALU.mult,
            )
            v.tensor_tensor(out=gn_row[:, :], in0=gn_row[:, :], in1=lng[:, :], op=ALU.mult)
            v.tensor_tensor(out=gn_row[:, :], in0=gn_row[:, :], in1=lnb[:, :], op=ALU.add)

            # ---- MLP layer 1: h = gelu(gn @ w1 + b1) ----
            ps_g = pps[0:HID, 96:288]
            nc.tensor.matmul(ps_g, ones1h[:, :], gn_row[:, :],
                             start=True, stop=True, skip_group_check=True)
            v.tensor_tensor(out=prod[:, :], in0=w1t[:, :], in1=ps_g, op=ALU.mult)
            v.reduce_sum(hvec[:, :], prod[:, :], axis=mybir.AxisListType.X)
            v.tensor_tensor(out=hvec[:, :], in0=hvec[:, :], in1=b1c[:, :], op=ALU.add)
            sc.activation(hvec[:, :], hvec[:, :], ACTF.Gelu)

            # ---- MLP layer 2 ----
            v.tensor_scalar(
                out=l2tmp[:, :], in0=w2t[:, :], scalar1=hvec[:, :], scalar2=None,
                op0=ALU.mult,
            )
            ps_z = pps[0:1, 288:291]
            nc.tensor.matmul(ps_z, ones_h1[:, :], l2tmp[:, :],
                             start=True, stop=True, skip_group_check=True)
            v.tensor_tensor(out=zrow[:, :], in0=ps_z, in1=b2r[:, :], op=ALU.add)

            # ---- softmax over 3 ----
            v.reduce_max(s1[:, :], zrow[:, :], axis=mybir.AxisListType.X)
            v.tensor_scalar(
                out=zrow[:, :], in0=zrow[:, :], scalar1=s1[:, :], scalar2=None,
                op0=ALU.subtract,
            )
            sc.activation(zrow[:, :], zrow[:, :], ACTF.Exp)
            v.reduce_sum(s2[:, :], zrow[:, :], axis=mybir.AxisListType.X)
            v.reciprocal(s2[:, :], s2[:, :])
            v.tensor_scalar(
                out=wts[:, :], in0=zrow[:, :], scalar1=s2[:, :], scalar2=None,
                op0=ALU.mult,
            )

            # ---- fold gate weights into per-tap channel weights ----
            ps_wb = pps[0:C, 291:294]
            nc.tensor.matmul(ps_wb, ones1c[:, :], wts[:, :],
                             start=True, stop=True, skip_group_check=True)
            v.tensor_scalar_mul(wts_bc[:, :], ps_wb, 1.0)
            for i in range(NB):
                v.tensor_scalar(
                    out=w_eff[0:C, i * 27 : (i + 1) * 27],
                    in0=cwt[:, i * 27 : (i + 1) * 27],
                    scalar1=wts_bc[:, i : i + 1],
                    scalar2=None,
                    op0=ALU.mult,
                )
            v.memset(w_eff[C : C + 1, :], 0.0)
            v.tensor_tensor(out=tmpb[:, :], in0=cbt[:, :], in1=wts_bc[:, :], op=ALU.mult)
            v.reduce_sum(b_eff[:, :], tmpb[:, :], axis=mybir.AxisListType.X)
            ctrs = [i * 27 + 13 for i in range(NB)]
            v.tensor_tensor(
                out=w_ctr[:, :], in0=w_eff[0:C, ctrs[0] : ctrs[0] + 1],
                in1=w_eff[0:C, ctrs[1] : ctrs[1] + 1], op=ALU.add,
            )
            v.tensor_tensor(
                out=w_ctr[:, :], in0=w_ctr[:, :],
                in1=w_eff[0:C, ctrs[2] : ctrs[2] + 1], op=ALU.add,
            )
            # b_eff as a row for the PE bias carrier (row 96 of dgB)
            ps_bt = pps[0:1, 294:390]
            nc.tensor.transpose(ps_bt, b_eff[:, :], idf32[:, :])

            # ---- PE diagonals ([97, C]; row 96 zero except bias carrier) ----
            diags = {}
            for t, od, oh, ow in asn["pe"]:
                dg = diagpool.tile([C + 1, C], F16, tag=f"dg{t}")
                v.tensor_scalar(
                    out=dg[:, :], in0=idp[:, :], scalar1=w_eff[:, t : t + 1],
                    scalar2=None, op0=ALU.mult,
                )
                diags[t] = dg
            bias_t = asn["pe"][0][0]
            v.tensor_scalar_mul(diags[bias_t][C : C + 1, :], ps_bt, 1.0)

            # ---- the conv ----
            def win(sidx, od, oh, ow):
                return XG + (sidx + od) * PS + oh * RS + ow

            def win3d(sidx, od, oh, ow, p0=0, p1=C):
                base = win(sidx, od, oh, ow)
                return xbuf[p0:p1, base : base + H * RS].rearrange(
                    "c (h w) -> c h w", h=H, w=RS
                )[:, :, 0:W]

            def winchunk(sidx, od, oh, ow, ci, p0=0, p1=C):
                base = win(sidx, od, oh, ow) + ci * 8 * RS
                return xbuf[p0:p1, base : base + 8 * RS].rearrange(
                    "c (h w) -> c h w", h=8, w=RS
                )[:, :, 0:W]

            def c3view(t_):
                return t_[:, :].rearrange("c (h w) -> c h w", h=H, w=W)

            for j in range(DH):
                sidx = S_OF + j
                pe_taps = asn["pe"]
                da_taps = asn["da"]
                ad_taps = asn["ad"]
                aa_taps = asn["aa"]

                # --- PE: chunk-major diagonal matmuls (bias tap first) ---
                pss = []
                for ci in range(NCH):
                    ps = ppool.tile([C, CCH], F32, tag=f"ps{ci}")
                    pss.append(ps)
                for ci in range(NCH):
                    for tn, (t, od, oh, ow) in enumerate(pe_taps):
                        nc.tensor.matmul(
                            pss[ci][:, :],
                            diags[t][:, :],
                            winchunk(sidx, od, oh, ow, ci, 0, C + 1),
                            start=(tn == 0),
                            stop=False,
                            skip_group_check=True,
                        )

                # --- Act products: first seeds c0, then tmpA tiles ---
                c0t = chpool.tile([C, PLANE], F16, tag="c0")
                t, od, oh, ow = ad_taps[0]
                sc.activation(c3view(c0t), win3d(sidx, od, oh, ow), ACTF.Copy,
                              scale=w_eff[0:C, t : t + 1])
                ad_tmp = []
                for t, od, oh, ow in ad_taps[1:]:
                    tt_ = tmpapool.tile([C, PLANE], F16, tag="tmpA")
                    sc.activation(c3view(tt_), win3d(sidx, od, oh, ow), ACTF.Copy,
                                  scale=w_eff[0:C, t : t + 1])
                    ad_tmp.append(tt_)
                aa_tmp = []
                for t, od, oh, ow in aa_taps:
                    tt_ = tmpapool.tile([C, PLANE], F16, tag="tmpA")
                    sc.activation(c3view(tt_), win3d(sidx, od, oh, ow), ACTF.Copy,
                                  scale=w_eff[0:C, t : t + 1])
                    aa_tmp.append(tt_)

                # --- DVE products + c0 adds + psum merges; gpsimd accum-DMA
                # chains c1..c3 consume the products ---
                nchain = min(3, len(da_taps))
                chains = []
                for m in range(nchain):
                    ch = chpool.tile([C, PLANE], F16, tag=f"c{m + 1}")
                    chains.append(ch)
                chain_last = [None] * nchain

                def emit_prod(k):
                    t, od, oh, ow = da_taps[k]
                    if k < nchain:
                        out_t = chains[k]
                        inst = v.tensor_scalar(
                            out=c3view(out_t), in0=win3d(sidx, od, oh, ow),
                            scalar1=w_eff[0:C, t : t + 1], scalar2=None, op0=ALU.mult,
                        )
                        chain_last[k] = inst
                    else:
                        tmp = tmpdpool.tile([C, PLANE], F16, tag="tmpD")
                        v.tensor_scalar(
                            out=c3view(tmp), in0=win3d(sidx, od, oh, ow),
                            scalar1=w_eff[0:C, t : t + 1], scalar2=None, op0=ALU.mult,
                        )
                        m = k % nchain
                        inst = nc.gpsimd.dma_start(
                            out=chains[m][:, :], in_=tmp[:, :], accum_op=ALU.add
                        )
                        dep(inst, chain_last[m])
                        chain_last[m] = inst

                nda = len(da_taps)
                kp = 0
                for _ in range(min(6, nda)):
                    emit_prod(kp)
                    kp += 1
                for ai, tt_ in enumerate(ad_tmp):
                    v.tensor_tensor(out=c0t[:, :], in0=c0t[:, :], in1=tt_[:, :], op=ALU.add)
                    if kp < nda:
                        emit_prod(kp)
                        kp += 1
                while kp < nda:
                    emit_prod(kp)
                    kp += 1
                # aa products join the DMA chains
                for i2, tt_ in enumerate(aa_tmp):
                    m = i2 % nchain
                    inst = nc.gpsimd.dma_start(
                        out=chains[m][:, :], in_=tt_[:, :], accum_op=ALU.add
                    )
                    dep(inst, chain_last[m])
                    chain_last[m] = inst

                # --- psum merges: a0[ci] = center*w_ctr + psum[ci] (fused) ---
                for ci in range(NCH):
                    v.scalar_tensor_tensor(
                        out=a0[:, ci * CCH : (ci + 1) * CCH].rearrange(
                            "c (h w) -> c h w", h=8, w=W),
                        in0=winchunk(sidx, 0, 0, 0, ci),
                        scalar=w_ctr[:, :],
                        in1=pss[ci][:, :].rearrange("c (h w) -> c h w", h=8, w=W),
                        op0=ALU.mult, op1=ALU.add,
                    )

                # --- Pool merges c0 (and chain 3) into a0; a0 seeds y; chains
                # 1/2 accumulate into y (serialized) ---
                nc.gpsimd.tensor_tensor(out=a0[:, :], in0=a0[:, :], in1=c0t[:, :], op=ALU.add)
                if nchain >= 3:
                    nc.gpsimd.tensor_tensor(
                        out=a0[:, :], in0=a0[:, :], in1=chains[2][:, :], op=ALU.add
                    )
                yreg = yout[:, j * PLANE : (j + 1) * PLANE]
                seed = nc.sync.dma_start(out=yreg, in_=a0[:, :])
                prev = seed
                for m in range(min(2, nchain)):
                    inst = nc.gpsimd.dma_start(
                        out=yreg, in_=chains[m][:, :], accum_op=ALU.add
                    )
                    dep(inst, prev)
                    if chain_last[m] is not None:
                        dep(inst, chain_last[m])
                    prev = inst

    _split_sem_waits(nc)
    return nc


_WAITSPLIT = [0]


def _split_sem_waits(nc, max_waits=1):
    """This walrus build rejects >1 SyncWait per instruction (and any wait on
    a Drain). Move excess waits onto same-engine NOPs inserted just before."""
    for bb in nc.main_func.blocks:
        insns = bb.instructions
        i = 0
        while i < len(insns):
            ins = insns[i]
            si = ins.sync_info
            limit = 0 if ins.opcode == "Drain" else max_waits
            if si is not None and si.on_wait is not None and len(si.on_wait) > limit:
                waits = list(si.on_wait)
                keep = waits[-limit:] if limit else []
                extra = waits[: len(waits) - limit]
                pos = i
                for j in range(0, len(extra), max_waits):
                    nop = mybir.InstNoOp(
                        name=f"I-waitsplit-{_WAITSPLIT[0]}", ins=[], outs=[]
                    )
                    _WAITSPLIT[0] += 1
                    nop.engine = ins.engine
                    nop.sync_info = mybir.SyncInfo(
                        on_wait=extra[j : j + max_waits], on_update=[]
                    )
                    insns.insert(pos, nop)
                    pos += 1
                    i += 1
                si.on_wait = keep
            i += 1


def _prep_inputs(x, guidance, convw, convb, ln_g, ln_b, w1, b1, w2, b2):
    f = np.float32
    cwt = np.ascontiguousarray(
        convw.reshape(NB, C, 27).transpose(1, 0, 2).reshape(C, NB * 27), dtype=f
    )
    cbt = np.ascontiguousarray(convb.T, dtype=f)
    w1t = np.ascontiguousarray(w1.T, dtype=f)
    idp = np.zeros((C + 1, C), dtype=np.float16)
    idp[:C] = np.eye(C, dtype=np.float16)
    idp[C] = 1.0
    common = dict(
        cwt=cwt, cbt=cbt, w1t=w1t,
        b1=np.ascontiguousarray(b1, dtype=f),
        w2=np.ascontiguousarray(w2, dtype=f),
        b2=np.ascontiguousarray(b2, dtype=f),
        lng=np.ascontiguousarray(ln_g, dtype=f),
        lnb=np.ascontiguousarray(ln_b, dtype=f),
        idp=idp,
    )
    x16 = x.astype(np.float16)
    in_maps = []
    for core in range(N_CORES):
        b, hh = core // 2, core % 2
        lo = DH * hh - S_OF  # global plane of slab idx 0 (may be <0 / >=D)
        buf = np.zeros((C + 1, XTOT), dtype=np.float16)
        buf[C, :] = 1.0
        view = buf[:C, XG:].reshape(C, NSLAB, PR, RS)
        g0s, g1s = max(0, lo), min(D, lo + NSLAB)
        view[:, g0s - lo : g1s - lo, :H, :W] = x16[b, :, g0s:g1s]
        if hh == 0:
            xplanes = x16[b, :, g1s:]       # planes 11..15
        else:
            xplanes = x16[b, :, :g0s]       # planes 0..4
        assert xplanes.shape[1] == NEXTRA
        xe = np.ascontiguousarray(xplanes.reshape(C, NEXTRA * PLANE))
        in_maps.append(
            dict(x=buf, xe=xe, gd=np.ascontiguousarray(guidance[b], dtype=f), **common)
        )
    return in_maps


_CACHED_NC = None


def kernel(x, guidance, convw, convb, ln_g, ln_b, w1, b1, w2, b2):
    global _CACHED_NC
    if _CACHED_NC is None:
        _CACHED_NC = _build_program()
    in_maps = _prep_inputs(
        x, guidance, convw, convb, ln_g, ln_b, w1, b1, w2, b2
    )
    res = run_bass_kernel_spmd(_CACHED_NC, in_maps, list(range(N_CORES)))
    out = np.empty((B, C, D, H, W), dtype=np.float32)
    for core in range(N_CORES):
        b, hh = core // 2, core % 2
        out[b, :, 8 * hh : 8 * hh + 8] = (
            res.results[core]["y"].astype(np.float32).reshape(C, DH, H, W)
        )
    return out


if __name__ == "__main__":
    rng = np.random.default_rng(0)
    ins = dict(
        x=rng.standard_normal((B, C, D, H, W), dtype=np.float32),
        guidance=rng.standard_normal((B, G), dtype=np.float32),
        convw=(rng.standard_normal((NB, C, 1, K, K, K)) * 0.1).astype(np.float32),
        convb=np.zeros((NB, C), np.float32),
        ln_g=np.ones((C + G,), np.float32),
        ln_b=np.zeros((C + G,), np.float32),
        w1=(rng.standard_normal((C + G, HID)) * 0.05).astype(np.float32),
        b1=np.zeros((HID,), np.float32),
        w2=(rng.standard_normal((HID, NB)) * 0.05).astype(np.float32),
        b2=np.zeros((NB,), np.float32),
    )
    out = kernel(**ins)
    print("kernel ran, out shape", out.shape)
